# revision 9
# baseline (speedup 1.0000x reference)
"""Sparse attention (template/search) Trainium2 kernel.

Model (per batch b):
  qkv = x @ qkv_w.T                  -> split to q, k, v heads (12 heads, hd=64)
  template tokens   [0, 256)  attend to template keys only
  search   tokens [256, 1280) attend to all 1280 keys
  out = softmax(q k^T / 8) v   per head, concat heads, @ proj_w.T + proj_b

Sharding: data-parallel over batch, one batch per NeuronCore (8 cores).
No collectives needed.

Layout strategy per core (v4):
  - every input tile DMA is split 4-way (32-row chunks) across queues: a
    single 128x768 fp32 tile on one queue takes ~26us (~200ns per 3KB row
    descriptor), which otherwise dominates the startup latency and the
    output tail.
  - x / qkv_w / proj_w are PE-transposed (fp32, the PSUM->SBUF copy casts
    to bf16) to xT [C, NTOK], wT [C, 3C], pwT [C, C].  DMA-priority order:
    x0, qk-weights, x1, v-weights -> pair-0 template scores run ~25us in
    (hoisted into the startup stream), search right after the v weights.
  - q,k computed feature-major into a 2-slot rotating buffer (slot =
    pair%2): qk[P, slot, {q,k}, NTOK] (q pre-scaled by 1/8).
  - v computed token-major, augmented per head as [1 | 63 zeros | v]:
    row 0 of the AV output is the softmax denominator.
  - scores computed TRANSPOSED: S.T[tk, tq] = K_h @ Q_h.T; the two heads
    of a pair fill the two halves of one [128, 1024] PSUM tile -> ONE exp
    instruction per (pair, cj, tk) covers both heads (N=1024 amortizes
    the ~300-cycle ACT instruction overhead).
  - search loop is cj-outer (two 512-token query chunks) so each head's
    AV accumulator is one PSUM bank: banks = 4 (scores, double-buffered)
    + 2 (AV accumulators) + 2 (fillers/transposes) = 8.
  - normalize fully off the ACT queue: DVE copy PSUM->SBUF, gpsimd
    partition_broadcast of row 0, DVE approx reciprocal, DVE multiply.
  - projection tiles stream out as soon as their inputs finalize: t0/t1
    during pair-5 cj 0 (template rows final after pair-5's template),
    t2-t5 during pair-5 cj 1, t6-t9 as the tail.

Scheduling: attention paces ACT(exp) and PE about evenly; all qkv / v /
proj-weight / early-proj work is software-pipelined into the search
loops as filler.  All matmuls bf16 (fp32 PSUM accumulation).
"""

import numpy as np

import concourse.bacc as bacc
import concourse.mybir as mybir
import concourse.tile as tile
from concourse.masks import make_identity

P = 128
NTOK = 1280
C = 768
H = 12
HD = 64
NT = 256          # template tokens  [0, NT)
TT = NTOK // P    # 10 token tiles
CT = C // P       # 6 channel tiles
SCALE = HD ** -0.5

F32 = mybir.dt.float32
BF16 = mybir.dt.bfloat16
EXP = mybir.ActivationFunctionType.Exp
MULT = mybir.AluOpType.mult
ADD = mybir.AluOpType.add


def build_nc():
    from contextlib import ExitStack

    nc = bacc.Bacc("TRN2", target_bir_lowering=False, debug=False, num_devices=8)
    x_ext = nc.dram_tensor("x", [NTOK, C], F32, kind="ExternalInput")
    w_ext = nc.dram_tensor("qkv_w", [3 * C, C], F32, kind="ExternalInput")
    pw_ext = nc.dram_tensor("proj_w", [C, C], F32, kind="ExternalInput")
    pb_ext = nc.dram_tensor("proj_b", [1, C], F32, kind="ExternalInput")
    out_ext = nc.dram_tensor("out", [NTOK, C], F32, kind="ExternalOutput")

    with tile.TileContext(nc) as tc, ExitStack() as ctx:
        const = ctx.enter_context(tc.tile_pool(name="const", bufs=1))
        big = ctx.enter_context(tc.tile_pool(name="big", bufs=1))
        # PSUM budget (16KB/partition): sc 2x4KB + ot 2x2KB + fill 2x2KB
        ps_sc = ctx.enter_context(tc.tile_pool(name="ps_sc", bufs=2, space="PSUM"))
        ps_ot = ctx.enter_context(tc.tile_pool(name="ps_ot", bufs=2, space="PSUM"))
        ps_fill = ctx.enter_context(tc.tile_pool(name="ps_fill", bufs=2, space="PSUM"))
        pts = ctx.enter_context(tc.tile_pool(name="pts", bufs=3))
        dn = ctx.enter_context(tc.tile_pool(name="dn", bufs=2))
        rbp = ctx.enter_context(tc.tile_pool(name="rbp", bufs=2))
        outp = ctx.enter_context(tc.tile_pool(name="outp", bufs=3))

        ident = const.tile([P, P], F32)
        make_identity(nc, ident)
        # HAM warmup: keep the PE busy during the initial input-DMA wait so
        # its clock gate opens (1.2 -> 2.4 GHz) before the real transpose
        # and qkv stream begins.
        warm_ps = ps_fill.tile([P, 512], F32, tag="fill")
        for i in range(24):
            nc.tensor.transpose(warm_ps[:, :P], ident[:], ident[:])
        nc.vector.tensor_copy(ident[:], warm_ps[:, :P])
        bias_bc = const.tile([P, C], F32)
        bias_row = const.tile([1, C], F32)
        nc.sync.dma_start(bias_row[:], pb_ext.ap())
        nc.gpsimd.partition_broadcast(bias_bc[:], bias_row[0:1, :])

        xT = big.tile([P, CT, NTOK], BF16)     # x.T  (feature-major x)
        wT = big.tile([P, CT, 3 * C], BF16)    # qkv_w.T
        pwT = big.tile([P, CT, C], BF16)       # proj_w.T
        pg = big.tile([P, CT, C], F32)         # proj_w fp32 (transposed p2/p3)

        def dma_in(dst_row, src, r0, nsplit=4):
            """Load ext rows [r0, r0+128) into a [128, C] tile AP, split
            across `nsplit` queues (single-queue 128-row tiles take ~26us)."""
            step = P // nsplit
            for s in range(nsplit):
                a, b = s * step, (s + 1) * step
                nc.sync.dma_start(dst_row[a:b, :], src[r0 + a:r0 + b, :])

        def transpose_blocks(srcs, dst_full):
            """PE-transpose fp32 [128,128] blocks through the filler PSUM
            ring; the PSUM->SBUF copy casts to bf16 (one copy per group)."""
            i = 0
            while i < len(srcs):
                n = min(4, len(srcs) - i)
                pt = ps_fill.tile([P, 512], F32, tag="fill")
                for j in range(n):
                    nc.tensor.transpose(
                        pt[:, j * P:(j + 1) * P], srcs[i + j], ident[:]
                    )
                nc.vector.tensor_copy(
                    dst_full[:, i * P:(i + n) * P], pt[:, : n * P]
                )
                i += n

        big2 = ctx.enter_context(tc.tile_pool(name="big2", bufs=1))
        # q (scaled) and k, feature-major, 2-slot rotation keyed by pair%2
        qk = big2.tile([P, 2, 2, NTOK], BF16)
        v_sb = big2.tile([P, TT, H, P], BF16)  # [1 | 63 zeros | v] per head
        ot_all = big2.tile([P, CT, NTOK], BF16)     # attention out, feature-major

        # v_aug layout per head: col 0 = ones (softmax denominator row),
        # cols 1:64 = zeros (padding so O lands at partitions 64:128)
        nc.gpsimd.memset(v_sb[:, :, :, 0:64], 0.0)
        nc.gpsimd.memset(v_sb[:, :, :, 0:1], 1.0)

        # ---- qkv projection pieces (emitted interleaved below) ----
        def emit_qk_chunk(hp, which, c0, cw):
            """qk[slot, which] = (q|k) row block of head pair hp,
            feature-major, for token chunk [c0, c0+cw)."""
            ft = hp + 6 * which
            ps = ps_fill.tile([P, 512], F32, tag="fill", name=f"qkp{ft}_{c0}")
            for ct in range(CT):
                nc.tensor.matmul(
                    ps[:, :cw],
                    wT[:, ct, ft * P:(ft + 1) * P],
                    xT[:, ct, c0:c0 + cw],
                    start=(ct == 0), stop=(ct == CT - 1),
                )
            if which == 0:  # q: fold in softmax scale
                nc.vector.tensor_scalar_mul(
                    qk[:, hp % 2, 0, c0:c0 + cw], ps[:, :cw], SCALE
                )
            else:
                nc.vector.tensor_copy(qk[:, hp % 2, 1, c0:c0 + cw], ps[:, :cw])

        def qk_pair_chunks(p):
            return [(p, w, c0, cw)
                    for c0, cw in ((0, 512), (512, 512), (1024, 256))
                    for w in (0, 1)]

        # v token-major: v[tok, f] = x @ qkv_w.T cols [1536, 2304)
        def emit_v_chunk(tt, half):
            c0, cw, h0, nh = ((0, 512, 0, 8), (512, 256, 8, 4))[half]
            ps = ps_fill.tile([P, 512], F32, tag="fill", name=f"vp{tt}_{half}")
            for ct in range(CT):
                nc.tensor.matmul(
                    ps[:, :cw],
                    xT[:, ct, tt * P:(tt + 1) * P],
                    wT[:, ct, 2 * C + c0:2 * C + c0 + cw],
                    start=(ct == 0), stop=(ct == CT - 1),
                )
            nc.vector.tensor_copy(
                v_sb[:, tt, h0:h0 + nh, 64:128],
                ps[:, :cw].rearrange("p (h e) -> p h e", e=HD),
            )

        # ---- output projection ----
        out_tiles = {}

        def emit_proj_chunk(tt, half):
            c0, cw = ((0, 512), (512, 256))[half]
            if half == 0:
                out_tiles[tt] = outp.tile([P, C], F32, tag="out",
                                          name=f"out{tt}")
            osb = out_tiles[tt]
            ps = ps_fill.tile([P, 512], F32, tag="fill", name=f"prj{tt}_{c0}")
            for ct in range(CT):
                nc.tensor.matmul(
                    ps[:, :cw],
                    ot_all[:, ct, tt * P:(tt + 1) * P],
                    pwT[:, ct, c0:c0 + cw],
                    start=(ct == 0), stop=(ct == CT - 1),
                )
            nc.vector.tensor_tensor(
                osb[:, c0:c0 + cw], ps[:, :cw], bias_bc[:, c0:c0 + cw], ADD,
            )
            if half == 1:
                t0 = tt * P
                for s in range(4):  # 4 queues so the last tile drains fast
                    a, b = s * 32, (s + 1) * 32
                    nc.sync.dma_start(out_ext.ap()[t0 + a:t0 + b, :],
                                      osb[a:b, :])

        def emit_filler(kind, arg):
            if kind == "qk":
                emit_qk_chunk(*arg)
            elif kind == "v":
                emit_v_chunk(*arg)
            elif kind == "proj":
                emit_proj_chunk(*arg)
            else:  # "pw": deferred proj_w transpose for channel tile arg
                transpose_blocks(
                    [pg[:, j, arg * P:(arg + 1) * P] for j in range(CT)],
                    pwT[:, arg, :],
                )

        # ---- attention helpers ----
        def qh(h, c0, cw):
            b = (h % 2) * 64
            return qk[b:b + 64, (h // 2) % 2, 0, c0:c0 + cw]

        def kh(h, tk):
            b = (h % 2) * 64
            return qk[b:b + 64, (h // 2) % 2, 1, tk * P:(tk + 1) * P]

        def normalize(h, ot_ps, c0, cw):
            """ot_ps: [128, cw] psum (row 0 = denominators, rows 64:128 = O.T
            for tq cols [c0, c0+cw)). Normalize and write to ot_all, fully
            off the ACT queue."""
            b = (h % 2) * 64
            den = dn.tile([P, 512], F32, tag="dn")
            nc.vector.tensor_copy(den[:, :cw], ot_ps[:, :cw])
            rb = rbp.tile([P, 512], F32, tag="rb")
            nc.gpsimd.partition_broadcast(rb[:, :cw], den[0:1, :cw])
            nc.vector.reciprocal_approx_fast(rb[:, :cw], rb[:, :cw])
            nc.vector.tensor_tensor(
                ot_all[b:b + 64, h // 2, c0:c0 + cw],
                den[64:128, :cw], rb[64:128, :cw], MULT,
            )

        def emit_template_scores(hp):
            h0, h1 = 2 * hp, 2 * hp + 1
            st_t = ps_sc.tile([P, 1024], F32, tag="sc", name=f"tst{hp}")
            for tj in range(2):
                for hi, h in enumerate((h0, h1)):
                    nc.tensor.matmul(
                        st_t[:, hi * 512 + tj * NT: hi * 512 + (tj + 1) * NT],
                        kh(h, tj), qh(h, 0, NT), start=True, stop=True,
                    )
            pt_t = pts.tile([P, 1024], BF16, tag="pt", name=f"tpt{hp}")
            nc.scalar.activation(pt_t[:], st_t[:], EXP)
            return pt_t

        def emit_template_av(hp, pt_t):
            for hi, h in enumerate((2 * hp, 2 * hp + 1)):
                to = ps_fill.tile([P, 512], F32, tag="fill", name=f"to{h}")
                for tj in range(2):
                    nc.tensor.matmul(
                        to[:, :NT], v_sb[:, tj, h, :],
                        pt_t[:, hi * 512 + tj * NT: hi * 512 + (tj + 1) * NT],
                        start=(tj == 0), stop=(tj == 1),
                    )
                normalize(h, to, 0, NT)

        # ---- startup: DMA-priority-ordered load + transpose ----
        with tc.tile_pool(name="staging", bufs=2) as staging:
            # x tokens 0-639
            xg0 = staging.tile([P, CT, C], F32, tag="g", name="xg0")
            for j in range(5):
                dma_in(xg0[:, j, :], x_ext.ap(), j * P)
            for ct in range(CT):
                transpose_blocks(
                    [xg0[:, j, ct * P:(ct + 1) * P] for j in range(5)],
                    xT[:, ct, 0:640],
                )
            # q weights (W rows 0-767), then k weights (rows 768-1535)
            for g in range(2):
                wg = staging.tile([P, CT, C], F32, tag="g", name=f"wg{g}")
                for j in range(CT):
                    dma_in(wg[:, j, :], w_ext.ap(), (g * 6 + j) * P)
                for ct in range(CT):
                    transpose_blocks(
                        [wg[:, j, ct * P:(ct + 1) * P] for j in range(CT)],
                        wT[:, ct, g * 6 * P:(g * 6 + 6) * P],
                    )
            # first q/k chunk of pair 0 -> template scores + exp start now
            emit_qk_chunk(0, 0, 0, 512)
            emit_qk_chunk(0, 1, 0, 512)
            pt_t0 = emit_template_scores(0)
            # x tokens 640-1279 (needed by the cj-0 search scores)
            xg1 = staging.tile([P, CT, C], F32, tag="g", name="xg1")
            for j in range(5):
                dma_in(xg1[:, j, :], x_ext.ap(), (5 + j) * P)
            for ct in range(CT):
                transpose_blocks(
                    [xg1[:, j, ct * P:(ct + 1) * P] for j in range(5)],
                    xT[:, ct, 640:1280],
                )
            for a in qk_pair_chunks(0)[2:]:
                emit_qk_chunk(*a)
            # v weights (W rows 1536-2303)
            wg2 = staging.tile([P, CT, C], F32, tag="g", name="wg2")
            for j in range(CT):
                dma_in(wg2[:, j, :], w_ext.ap(), (12 + j) * P)
            for ct in range(CT):
                transpose_blocks(
                    [wg2[:, j, ct * P:(ct + 1) * P] for j in range(CT)],
                    wT[:, ct, 2 * C:3 * C],
                )
            for tt in (0, 1):
                emit_v_chunk(tt, 0)
                emit_v_chunk(tt, 1)
            emit_template_av(0, pt_t0)
            # proj_w last (transposed as pair-2/3 filler)
            for j in range(CT):
                dma_in(pg[:, j, :], pw_ext.ap(), j * P, nsplit=2)

        # ---- attention main loop ----
        for hp in range(6):
            h0, h1 = 2 * hp, 2 * hp + 1
            if hp == 0:
                pend = [[("v", (tt, half)) for tt in range(2, TT)
                         for half in (0, 1)],
                        [("qk", a) for a in qk_pair_chunks(1)]]
            elif hp in (1, 4):
                nxt = [("qk", a) for a in qk_pair_chunks(hp + 1)]
                pend = [nxt[:3], nxt[3:]]
            elif hp in (2, 3):
                nxt = [("qk", a) for a in qk_pair_chunks(hp + 1)]
                pend = [nxt[:3], nxt[3:] + [("pw", ct) for ct in
                                            range(0 if hp == 2 else 3,
                                                  3 if hp == 2 else CT)]]
            else:  # pair 5: template tokens final after its template below
                pend = [[("proj", (tt, half)) for tt in (0, 1)
                         for half in (0, 1)], []]

            if hp >= 1:
                pt_t = emit_template_scores(hp)
                emit_template_av(hp, pt_t)

            for cj in range(2):
                c0 = NT + cj * 512
                pending = pend[cj]
                ots = {h: ps_ot.tile([P, 512], F32, tag="ot",
                                     name=f"ot{h}_{cj}")
                       for h in (h0, h1)}
                for tk in range(TT):
                    st = ps_sc.tile([P, 1024], F32, tag="sc",
                                    name=f"st{hp}_{cj}_{tk}")
                    for hi, h in enumerate((h0, h1)):
                        nc.tensor.matmul(
                            st[:, hi * 512:(hi + 1) * 512],
                            kh(h, tk), qh(h, c0, 512), start=True, stop=True,
                        )
                    pt = pts.tile([P, 1024], BF16, tag="pt",
                                  name=f"pt{hp}_{cj}_{tk}")
                    nc.scalar.activation(pt[:], st[:], EXP)
                    for hi, h in enumerate((h0, h1)):
                        nc.tensor.matmul(
                            ots[h][:, :], v_sb[:, tk, h, :],
                            pt[:, hi * 512:(hi + 1) * 512],
                            start=(tk == 0), stop=(tk == TT - 1),
                        )
                    for _ in range(2 if (hp == 0 and cj == 0) else 1):
                        if pending:
                            emit_filler(*pending.pop(0))
                for h in (h0, h1):
                    normalize(h, ots[h], c0, 512)
                while pending:
                    emit_filler(*pending.pop(0))
                if hp == 5 and cj == 0:
                    # queries 256-767 (tiles 2-5) final after every pair's
                    # cj-0 normalize -> proj them during cj 1
                    pend[1] = [("proj", (tt, half)) for tt in (2, 3, 4, 5)
                               for half in (0, 1)]

        # tail: only tiles 6-9 remain (they need pair-5 cj-1)
        for tt in (6, 7, 8, 9):
            for half in (0, 1):
                emit_proj_chunk(tt, half)

    nc.compile()
    return nc


_NC = None


def _get_nc():
    global _NC
    if _NC is None:
        _NC = build_nc()
    return _NC


def kernel(x, qkv_w, proj_w, proj_b, **_ignored):
    from concourse.bass_utils import run_bass_kernel_spmd

    x = np.ascontiguousarray(np.asarray(x), dtype=np.float32)
    qkv_w = np.ascontiguousarray(np.asarray(qkv_w), dtype=np.float32)
    proj_w = np.ascontiguousarray(np.asarray(proj_w), dtype=np.float32)
    proj_b = np.ascontiguousarray(np.asarray(proj_b), dtype=np.float32).reshape(1, C)

    nc = _get_nc()
    in_maps = [
        {"x": x[i], "qkv_w": qkv_w, "proj_w": proj_w, "proj_b": proj_b}
        for i in range(8)
    ]
    res = run_bass_kernel_spmd(nc, in_maps, list(range(8)))
    return np.stack([res.results[i]["out"] for i in range(8)])


if __name__ == "__main__":
    rng = np.random.default_rng(0)
    ins = {
        "x": rng.standard_normal((8, NTOK, C), dtype=np.float32),
        "qkv_w": rng.standard_normal((3 * C, C), dtype=np.float32) * 0.02,
        "proj_w": rng.standard_normal((C, C), dtype=np.float32) * 0.02,
        "proj_b": np.zeros(C, dtype=np.float32),
    }
    out = kernel(**ins)
    print("out", out.shape, out.dtype)


# revision 11
# speedup vs baseline: 1.2901x; 1.2901x over previous
"""Sparse attention (template/search) Trainium2 kernel.

Model (per batch b):
  qkv = x @ qkv_w.T                  -> split to q, k, v heads (12 heads, hd=64)
  template tokens   [0, 256)  attend to template keys only
  search   tokens [256, 1280) attend to all 1280 keys
  out = softmax(q k^T / 8) v   per head, concat heads, @ proj_w.T + proj_b

Sharding: data-parallel over batch, one batch per NeuronCore (8 cores).
No collectives needed.

Layout strategy per core (v5):
  - input DMA is both issue-limited (~630ns per dma_start on the Sync
    queue) and descriptor-limited (~200ns per 3KB row on a queue), so the
    startup loads exactly what the first attention pair needs, in need
    order: x tokens 0-639, the q0/k0 weight rows, x tokens 640-1279, the
    v weights.  The q/k weight rows for pairs 1-5 stream in afterwards
    and are transposed inside the attention loop as filler work.
  - x / qkv_w / proj_w are PE-transposed (fp32, the PSUM->SBUF copy casts
    to bf16) to xT [C, NTOK], wT [C, 3C], pwT [C, C].  Deferred-weight
    transposes write through strided 3D destination APs (one DVE copy per
    PSUM group) so they do not clog the 2-buffer filler PSUM ring.
  - q,k computed feature-major into a 2-slot rotating buffer (slot =
    pair%2): qk[P, slot, {q,k}, NTOK] (q pre-scaled by 1/8).
  - v computed token-major, augmented per head as [1 | 63 zeros | v]:
    row 0 of the AV output is the softmax denominator.
  - scores computed TRANSPOSED: S.T[tk, tq] = K_h @ Q_h.T; the two heads
    of a pair fill the two halves of one [128, 1024] PSUM tile -> ONE exp
    instruction per (pair, cj, tk) covers both heads (N=1024 amortizes
    the ~300-cycle ACT instruction overhead).
  - search loop is cj-outer (two 512-token query chunks) so each head's
    AV accumulator is one PSUM bank: banks = 4 (scores, double-buffered)
    + 2 (AV accumulators) + 2 (fillers/transposes) = 8.
  - pair 0's template block is hoisted into the startup stream (its
    scores need only the first q/k chunk) so the ACT exp pipeline starts
    ~25us in.
  - normalize fully off the ACT queue: DVE copy PSUM->SBUF, gpsimd
    partition_broadcast of row 0, DVE approx reciprocal, DVE multiply.
  - projection tiles stream out as soon as their inputs finalize: t0/t1
    during pair-5 cj 0 (template rows final after pair-5's template),
    t2-t5 during pair-5 cj 1, t6-t9 as the tail; output DMAs are split
    4-way across queues so the last tile drains in ~7us, not 26.

Scheduling: attention paces ACT(exp) and PE about evenly; all qkv / v /
deferred-weight-transpose / proj work is software-pipelined into the
search loops as filler.  All matmuls bf16 (fp32 PSUM accumulation).
"""

import numpy as np

import concourse.bacc as bacc
import concourse.mybir as mybir
import concourse.tile as tile
from concourse.masks import make_identity

P = 128
NTOK = 1280
C = 768
H = 12
HD = 64
NT = 256          # template tokens  [0, NT)
TT = NTOK // P    # 10 token tiles
CT = C // P       # 6 channel tiles
SCALE = HD ** -0.5

F32 = mybir.dt.float32
BF16 = mybir.dt.bfloat16
EXP = mybir.ActivationFunctionType.Exp
MULT = mybir.AluOpType.mult
ADD = mybir.AluOpType.add


def build_nc():
    from contextlib import ExitStack

    nc = bacc.Bacc("TRN2", target_bir_lowering=False, debug=False, num_devices=8)
    x_ext = nc.dram_tensor("x", [NTOK, C], F32, kind="ExternalInput")
    w_ext = nc.dram_tensor("qkv_w", [3 * C, C], F32, kind="ExternalInput")
    pw_ext = nc.dram_tensor("proj_w", [C, C], F32, kind="ExternalInput")
    pb_ext = nc.dram_tensor("proj_b", [1, C], F32, kind="ExternalInput")
    out_ext = nc.dram_tensor("out", [NTOK, C], F32, kind="ExternalOutput")

    with tile.TileContext(nc) as tc, ExitStack() as ctx:
        const = ctx.enter_context(tc.tile_pool(name="const", bufs=1))
        big = ctx.enter_context(tc.tile_pool(name="big", bufs=1))
        staging = ctx.enter_context(tc.tile_pool(name="staging", bufs=2))
        # PSUM budget (16KB/partition): sc 2x4KB + ot 2x2KB + fill 2x2KB
        ps_sc = ctx.enter_context(tc.tile_pool(name="ps_sc", bufs=2, space="PSUM"))
        ps_ot = ctx.enter_context(tc.tile_pool(name="ps_ot", bufs=2, space="PSUM"))
        ps_fill = ctx.enter_context(tc.tile_pool(name="ps_fill", bufs=2, space="PSUM"))
        pts = ctx.enter_context(tc.tile_pool(name="pts", bufs=3))
        dn = ctx.enter_context(tc.tile_pool(name="dn", bufs=2))
        rbp = ctx.enter_context(tc.tile_pool(name="rbp", bufs=2))
        outp = ctx.enter_context(tc.tile_pool(name="outp", bufs=3))

        ident = const.tile([P, P], F32)
        make_identity(nc, ident)
        # HAM warmup: keep the PE busy during the initial input-DMA wait so
        # its clock gate opens (1.2 -> 2.4 GHz) before the real transpose
        # and qkv stream begins.
        warm_ps = ps_fill.tile([P, 512], F32, tag="fill")
        for i in range(24):
            nc.tensor.transpose(warm_ps[:, :P], ident[:], ident[:])
        nc.vector.tensor_copy(ident[:], warm_ps[:, :P])
        bias_bc = const.tile([P, C], F32)
        bias_row = const.tile([1, C], F32)
        nc.sync.dma_start(bias_row[:], pb_ext.ap())
        nc.gpsimd.partition_broadcast(bias_bc[:], bias_row[0:1, :])

        xT = big.tile([P, CT, NTOK], BF16)     # x.T  (feature-major x)
        wT = big.tile([P, CT, 3 * C], BF16)    # qkv_w.T
        pwT = big.tile([P, CT, C], BF16)       # proj_w.T
        pg = big.tile([P, CT, C], F32)         # proj_w fp32 (transposed p2-p4)

        def transpose_blocks(srcs, dst_full):
            """PE-transpose fp32 [128,128] blocks through the filler PSUM
            ring; the PSUM->SBUF copy casts to bf16 (one copy per group)."""
            i = 0
            while i < len(srcs):
                n = min(4, len(srcs) - i)
                pt = ps_fill.tile([P, 512], F32, tag="fill")
                for j in range(n):
                    nc.tensor.transpose(
                        pt[:, j * P:(j + 1) * P], srcs[i + j], ident[:]
                    )
                nc.vector.tensor_copy(
                    dst_full[:, i * P:(i + n) * P], pt[:, : n * P]
                )
                i += n

        def transpose_wcol(src_row, fcol):
            """Transpose one qkv_w row block [128, 768] into wT's feature
            column fcol*128 across all 6 channel tiles.  The destination is
            strided (one 128-col block per channel tile), written with a
            single 3D-AP DVE copy per PSUM group."""
            for ct0, nct in ((0, 4), (4, 2)):
                pt = ps_fill.tile([P, 512], F32, tag="fill")
                for j in range(nct):
                    nc.tensor.transpose(
                        pt[:, j * P:(j + 1) * P],
                        src_row[:, (ct0 + j) * P:(ct0 + j + 1) * P], ident[:]
                    )
                nc.vector.tensor_copy(
                    wT[:, ct0:ct0 + nct, fcol * P:(fcol + 1) * P],
                    pt[:, :nct * P].rearrange("p (a b) -> p a b", b=P),
                )

        big2 = ctx.enter_context(tc.tile_pool(name="big2", bufs=1))
        # q (scaled) and k, feature-major, 2-slot rotation keyed by pair%2
        qk = big2.tile([P, 2, 2, NTOK], BF16)
        v_sb = big2.tile([P, TT, H, P], BF16)  # [1 | 63 zeros | v] per head
        ot_all = big2.tile([P, CT, NTOK], BF16)     # attention out, feature-major

        # v_aug layout per head: col 0 = ones (softmax denominator row),
        # cols 1:64 = zeros (padding so O lands at partitions 64:128)
        nc.gpsimd.memset(v_sb[:, :, :, 0:64], 0.0)
        nc.gpsimd.memset(v_sb[:, :, :, 0:1], 1.0)

        # ---- qkv projection pieces (emitted interleaved below) ----
        def emit_qk_chunk(hp, which, c0, cw):
            """qk[slot, which] = (q|k) row block of head pair hp,
            feature-major, for token chunk [c0, c0+cw)."""
            ft = hp + 6 * which
            ps = ps_fill.tile([P, 512], F32, tag="fill", name=f"qkp{ft}_{c0}")
            for ct in range(CT):
                nc.tensor.matmul(
                    ps[:, :cw],
                    wT[:, ct, ft * P:(ft + 1) * P],
                    xT[:, ct, c0:c0 + cw],
                    start=(ct == 0), stop=(ct == CT - 1),
                )
            if which == 0:  # q: fold in softmax scale
                nc.vector.tensor_scalar_mul(
                    qk[:, hp % 2, 0, c0:c0 + cw], ps[:, :cw], SCALE
                )
            else:
                nc.vector.tensor_copy(qk[:, hp % 2, 1, c0:c0 + cw], ps[:, :cw])

        def qk_pair_chunks(p):
            return [(p, w, c0, cw)
                    for c0, cw in ((0, 512), (512, 512), (1024, 256))
                    for w in (0, 1)]

        # v token-major: v[tok, f] = x @ qkv_w.T cols [1536, 2304)
        def emit_v_chunk(tt, half):
            c0, cw, h0, nh = ((0, 512, 0, 8), (512, 256, 8, 4))[half]
            ps = ps_fill.tile([P, 512], F32, tag="fill", name=f"vp{tt}_{half}")
            for ct in range(CT):
                nc.tensor.matmul(
                    ps[:, :cw],
                    xT[:, ct, tt * P:(tt + 1) * P],
                    wT[:, ct, 2 * C + c0:2 * C + c0 + cw],
                    start=(ct == 0), stop=(ct == CT - 1),
                )
            nc.vector.tensor_copy(
                v_sb[:, tt, h0:h0 + nh, 64:128],
                ps[:, :cw].rearrange("p (h e) -> p h e", e=HD),
            )

        # ---- output projection ----
        out_tiles = {}

        def emit_proj_chunk(tt, half):
            c0, cw = ((0, 512), (512, 256))[half]
            if half == 0:
                out_tiles[tt] = outp.tile([P, C], F32, tag="out",
                                          name=f"out{tt}")
            osb = out_tiles[tt]
            ps = ps_fill.tile([P, 512], F32, tag="fill", name=f"prj{tt}_{c0}")
            for ct in range(CT):
                nc.tensor.matmul(
                    ps[:, :cw],
                    ot_all[:, ct, tt * P:(tt + 1) * P],
                    pwT[:, ct, c0:c0 + cw],
                    start=(ct == 0), stop=(ct == CT - 1),
                )
            nc.vector.tensor_tensor(
                osb[:, c0:c0 + cw], ps[:, :cw], bias_bc[:, c0:c0 + cw], ADD,
            )
            if half == 1:
                t0 = tt * P
                for s in range(4):  # 4 queues so the last tile drains fast
                    a, b = s * 32, (s + 1) * 32
                    nc.sync.dma_start(out_ext.ap()[t0 + a:t0 + b, :],
                                      osb[a:b, :])

        wrest = {}  # deferred q/k weight rows j1-j5, per group

        def emit_filler(kind, arg):
            if kind == "qk":
                emit_qk_chunk(*arg)
            elif kind == "v":
                emit_v_chunk(*arg)
            elif kind == "proj":
                emit_proj_chunk(*arg)
            elif kind == "wt":
                g, j = arg
                transpose_wcol(wrest[g][:, j - 1, :], g * 6 + j)
            else:  # "pw": deferred proj_w transpose for channel tile arg
                transpose_blocks(
                    [pg[:, j, arg * P:(arg + 1) * P] for j in range(CT)],
                    pwT[:, arg, :],
                )

        # ---- attention helpers ----
        def qh(h, c0, cw):
            b = (h % 2) * 64
            return qk[b:b + 64, (h // 2) % 2, 0, c0:c0 + cw]

        def kh(h, tk):
            b = (h % 2) * 64
            return qk[b:b + 64, (h // 2) % 2, 1, tk * P:(tk + 1) * P]

        def normalize(h, ot_ps, c0, cw):
            """ot_ps: [128, cw] psum (row 0 = denominators, rows 64:128 = O.T
            for tq cols [c0, c0+cw)). Normalize and write to ot_all, fully
            off the ACT queue."""
            b = (h % 2) * 64
            den = dn.tile([P, 512], F32, tag="dn")
            nc.vector.tensor_copy(den[:, :cw], ot_ps[:, :cw])
            rb = rbp.tile([P, 512], F32, tag="rb")
            nc.gpsimd.partition_broadcast(rb[:, :cw], den[0:1, :cw])
            nc.vector.reciprocal_approx_fast(rb[:, :cw], rb[:, :cw])
            nc.vector.tensor_tensor(
                ot_all[b:b + 64, h // 2, c0:c0 + cw],
                den[64:128, :cw], rb[64:128, :cw], MULT,
            )

        def emit_template_scores(hp):
            h0, h1 = 2 * hp, 2 * hp + 1
            st_t = ps_sc.tile([P, 1024], F32, tag="sc", name=f"tst{hp}")
            for tj in range(2):
                for hi, h in enumerate((h0, h1)):
                    nc.tensor.matmul(
                        st_t[:, hi * 512 + tj * NT: hi * 512 + (tj + 1) * NT],
                        kh(h, tj), qh(h, 0, NT), start=True, stop=True,
                    )
            pt_t = pts.tile([P, 1024], BF16, tag="pt", name=f"tpt{hp}")
            nc.scalar.activation(pt_t[:], st_t[:], EXP)
            return pt_t

        def emit_template_av(hp, pt_t):
            for hi, h in enumerate((2 * hp, 2 * hp + 1)):
                to = ps_fill.tile([P, 512], F32, tag="fill", name=f"to{h}")
                for tj in range(2):
                    nc.tensor.matmul(
                        to[:, :NT], v_sb[:, tj, h, :],
                        pt_t[:, hi * 512 + tj * NT: hi * 512 + (tj + 1) * NT],
                        start=(tj == 0), stop=(tj == 1),
                    )
                normalize(h, to, 0, NT)

        # ---- startup: DMA-priority-ordered load + transpose ----
        # x tokens 0-639
        xg0 = staging.tile([P, CT, C], F32, tag="g", name="xg0")
        for j in range(5):
            nc.sync.dma_start(xg0[:, j, :], x_ext.ap()[j * P:(j + 1) * P, :])
        for ct in range(CT):
            transpose_blocks(
                [xg0[:, j, ct * P:(ct + 1) * P] for j in range(5)],
                xT[:, ct, 0:640],
            )
        # q0 / k0 weight rows (W rows 0-127 and 768-895)
        wj0 = staging.tile([P, CT, C], F32, tag="g", name="wj0")
        nc.sync.dma_start(wj0[:, 0, :], w_ext.ap()[0:P, :])
        nc.sync.dma_start(wj0[:, 1, :], w_ext.ap()[6 * P:7 * P, :])
        transpose_wcol(wj0[:, 0, :], 0)
        transpose_wcol(wj0[:, 1, :], 6)
        # first q/k chunk of pair 0 -> template scores + exp start now
        emit_qk_chunk(0, 0, 0, 512)
        emit_qk_chunk(0, 1, 0, 512)
        pt_t0 = emit_template_scores(0)
        # x tokens 640-1279 (needed by the cj-0 search scores)
        xg1 = staging.tile([P, CT, C], F32, tag="g", name="xg1")
        for j in range(5):
            t0 = (5 + j) * P
            nc.sync.dma_start(xg1[:, j, :], x_ext.ap()[t0:t0 + P, :])
        for ct in range(CT):
            transpose_blocks(
                [xg1[:, j, ct * P:(ct + 1) * P] for j in range(5)],
                xT[:, ct, 640:1280],
            )
        for a in qk_pair_chunks(0)[2:]:
            emit_qk_chunk(*a)
        # v weights (W rows 1536-2303)
        wg2 = staging.tile([P, CT, C], F32, tag="g", name="wg2")
        for j in range(CT):
            nc.sync.dma_start(wg2[:, j, :],
                              w_ext.ap()[(12 + j) * P:(13 + j) * P, :])
        for ct in range(CT):
            transpose_blocks(
                [wg2[:, j, ct * P:(ct + 1) * P] for j in range(CT)],
                wT[:, ct, 2 * C:3 * C],
            )
        for tt in (0, 1):
            emit_v_chunk(tt, 0)
            emit_v_chunk(tt, 1)
        emit_template_av(0, pt_t0)
        # deferred q/k weight rows j1-5 (transposed as pair 0-3 fillers)
        for g in range(2):
            wrest[g] = staging.tile([P, CT, C], F32, tag="g", name=f"wr{g}")
            for j in range(1, 6):
                nc.sync.dma_start(wrest[g][:, j - 1, :],
                                  w_ext.ap()[(g * 6 + j) * P:
                                             (g * 6 + j + 1) * P, :])
        # proj_w last (transposed as pair-2/3/4 filler)
        for j in range(CT):
            nc.sync.dma_start(pg[:, j, :], pw_ext.ap()[j * P:(j + 1) * P, :])

        # ---- attention main loop ----
        for hp in range(6):
            h0, h1 = 2 * hp, 2 * hp + 1
            # filler schedule: weight row j is due before pair j's template;
            # v tiles stream JIT inside pair 0 cj 0; proj_w transposes are
            # due before pair-5 cj 0 (early proj tiles)
            if hp == 0:
                pend = [[("v", (tt, half)) for tt in range(2, TT)
                         for half in (0, 1)],
                        [("wt", (0, 1)), ("wt", (1, 1))]
                        + [("qk", a) for a in qk_pair_chunks(1)]]
            elif hp == 1:
                nxt = [("qk", a) for a in qk_pair_chunks(2)]
                pend = [[("wt", (0, 2)), ("wt", (1, 2))] + nxt[:2],
                        nxt[2:] + [("wt", (0, 3)), ("wt", (1, 3))]]
            elif hp == 2:
                nxt = [("qk", a) for a in qk_pair_chunks(3)]
                pend = [[("wt", (0, 4))] + nxt[:3],
                        nxt[3:] + [("wt", (1, 4)), ("pw", 0)]]
            elif hp == 3:
                nxt = [("qk", a) for a in qk_pair_chunks(4)]
                pend = [[("wt", (0, 5))] + nxt[:3],
                        nxt[3:] + [("wt", (1, 5)), ("pw", 1), ("pw", 2)]]
            elif hp == 4:
                nxt = [("qk", a) for a in qk_pair_chunks(5)]
                pend = [nxt[:3] + [("pw", 3)],
                        nxt[3:] + [("pw", 4), ("pw", 5)]]
            else:  # pair 5: template tokens final after its template below
                pend = [[("proj", (tt, half)) for tt in (0, 1)
                         for half in (0, 1)], []]

            if hp >= 1:
                pt_t = emit_template_scores(hp)
                emit_template_av(hp, pt_t)

            for cj in range(2):
                c0 = NT + cj * 512
                pending = pend[cj]
                ots = {h: ps_ot.tile([P, 512], F32, tag="ot",
                                     name=f"ot{h}_{cj}")
                       for h in (h0, h1)}
                for tk in range(TT):
                    st = ps_sc.tile([P, 1024], F32, tag="sc",
                                    name=f"st{hp}_{cj}_{tk}")
                    for hi, h in enumerate((h0, h1)):
                        nc.tensor.matmul(
                            st[:, hi * 512:(hi + 1) * 512],
                            kh(h, tk), qh(h, c0, 512), start=True, stop=True,
                        )
                    pt = pts.tile([P, 1024], BF16, tag="pt",
                                  name=f"pt{hp}_{cj}_{tk}")
                    nc.scalar.activation(pt[:], st[:], EXP)
                    for hi, h in enumerate((h0, h1)):
                        nc.tensor.matmul(
                            ots[h][:, :], v_sb[:, tk, h, :],
                            pt[:, hi * 512:(hi + 1) * 512],
                            start=(tk == 0), stop=(tk == TT - 1),
                        )
                    for _ in range(2 if (hp == 0 and cj == 0) else 1):
                        if pending:
                            emit_filler(*pending.pop(0))
                for h in (h0, h1):
                    normalize(h, ots[h], c0, 512)
                while pending:
                    emit_filler(*pending.pop(0))
                if hp == 5 and cj == 0:
                    # queries 256-767 (tiles 2-5) final after every pair's
                    # cj-0 normalize -> proj them during cj 1
                    pend[1] = [("proj", (tt, half)) for tt in (2, 3, 4, 5)
                               for half in (0, 1)]

        # tail: only tiles 6-9 remain (they need pair-5 cj-1)
        for tt in (6, 7, 8, 9):
            for half in (0, 1):
                emit_proj_chunk(tt, half)

    nc.compile()
    return nc


_NC = None


def _get_nc():
    global _NC
    if _NC is None:
        _NC = build_nc()
    return _NC


def kernel(x, qkv_w, proj_w, proj_b, **_ignored):
    from concourse.bass_utils import run_bass_kernel_spmd

    x = np.ascontiguousarray(np.asarray(x), dtype=np.float32)
    qkv_w = np.ascontiguousarray(np.asarray(qkv_w), dtype=np.float32)
    proj_w = np.ascontiguousarray(np.asarray(proj_w), dtype=np.float32)
    proj_b = np.ascontiguousarray(np.asarray(proj_b), dtype=np.float32).reshape(1, C)

    nc = _get_nc()
    in_maps = [
        {"x": x[i], "qkv_w": qkv_w, "proj_w": proj_w, "proj_b": proj_b}
        for i in range(8)
    ]
    res = run_bass_kernel_spmd(nc, in_maps, list(range(8)))
    return np.stack([res.results[i]["out"] for i in range(8)])


if __name__ == "__main__":
    rng = np.random.default_rng(0)
    ins = {
        "x": rng.standard_normal((8, NTOK, C), dtype=np.float32),
        "qkv_w": rng.standard_normal((3 * C, C), dtype=np.float32) * 0.02,
        "proj_w": rng.standard_normal((C, C), dtype=np.float32) * 0.02,
        "proj_b": np.zeros(C, dtype=np.float32),
    }
    out = kernel(**ins)
    print("out", out.shape, out.dtype)


# revision 16
# speedup vs baseline: 1.3882x; 1.0760x over previous
"""Sparse attention (template/search) Trainium2 kernel.

Model (per batch b):
  qkv = x @ qkv_w.T                  -> split to q, k, v heads (12 heads, hd=64)
  template tokens   [0, 256)  attend to template keys only
  search   tokens [256, 1280) attend to all 1280 keys
  out = softmax(q k^T / 8) v   per head, concat heads, @ proj_w.T + proj_b

Sharding: data-parallel over batch, one batch per NeuronCore (8 cores).
No collectives needed.

Layout strategy per core (v2):
  - x / qkv_w are cast fp32->bf16 on DVE right after DMA, then PE-transposed
    in bf16 (1 cyc/col vs ~4 for fp32) to xT [C, NTOK], wT [C, 3C].  The
    transpose PSUM tiles are bf16 so the PSUM->SBUF copies run in the DVE
    16-bit packed mode.  (proj_w keeps the fp32 transpose path: its PSUM
    tiles borrow the filler ring, which is fp32.)
  - q,k computed feature-major into a 2-slot rotating buffer (slot =
    pair%2): qk[P, slot, {q,k}, NTOK] (q pre-scaled by 1/8).
  - v computed token-major, augmented per head as [1 | 63 zeros | v]:
    row 0 of the AV output is the softmax denominator.
  - scores computed TRANSPOSED: S.T[tk, tq] = K_h @ Q_h.T.  The two heads
    of a pair sit on PE row groups 0-63 / 64-127, so their score matmuls
    run CONCURRENTLY (tile_position row packing), filling the two halves
    of one [128, 1024] PSUM tile -> ONE exp instruction per (pair, cj, tk)
    covers both heads (N=1024 amortizes the ~300-cycle ACT overhead).
  - search loop is cj-outer (two 512-token query chunks) so each head's
    AV accumulator is one PSUM bank: banks = 4 (scores, double-buffered)
    + 2 (AV accumulators) + 2 (qkv/proj fillers) = 8.
  - normalize fully off the ACT queue: DVE copy PSUM->SBUF, gpsimd
    partition_broadcast of row 0, DVE approx reciprocal, DVE multiply.
  - proj: out[tok, c] = ot_all.T @ pwT; token tiles 2-5 (queries 256-767)
    are emitted as fillers inside the last pair's cj=1 loop (their inputs
    finalize at cj=0 normalize), shrinking the serial tail.

Scheduling: attention paces ACT(exp) and PE about evenly; all qkv / v /
proj-weight work is software-pipelined into the search loops as filler.
All matmuls bf16 (fp32 PSUM accumulation).
"""

import numpy as np

import concourse.bacc as bacc
import concourse.mybir as mybir
import concourse.tile as tile
from concourse.masks import make_identity

P = 128
NTOK = 1280
C = 768
H = 12
HD = 64
NT = 256          # template tokens  [0, NT)
TT = NTOK // P    # 10 token tiles
CT = C // P       # 6 channel tiles
SCALE = HD ** -0.5

F32 = mybir.dt.float32
BF16 = mybir.dt.bfloat16
EXP = mybir.ActivationFunctionType.Exp
MULT = mybir.AluOpType.mult
ADD = mybir.AluOpType.add


def build_nc():
    from contextlib import ExitStack

    nc = bacc.Bacc("TRN2", target_bir_lowering=False, debug=False, num_devices=8)
    x_ext = nc.dram_tensor("x", [NTOK, C], F32, kind="ExternalInput")
    w_ext = nc.dram_tensor("qkv_w", [3 * C, C], F32, kind="ExternalInput")
    pw_ext = nc.dram_tensor("proj_w", [C, C], F32, kind="ExternalInput")
    pb_ext = nc.dram_tensor("proj_b", [1, C], F32, kind="ExternalInput")
    out_ext = nc.dram_tensor("out", [NTOK, C], F32, kind="ExternalOutput")

    with tile.TileContext(nc) as tc, ExitStack() as ctx:
        const = ctx.enter_context(tc.tile_pool(name="const", bufs=1))
        big = ctx.enter_context(tc.tile_pool(name="big", bufs=1))

        identb = const.tile([P, P], BF16)
        make_identity(nc, identb)
        bias_bc = const.tile([P, C], F32)
        bias_row = const.tile([1, C], F32)
        nc.sync.dma_start(bias_row[:], pb_ext.ap())
        nc.gpsimd.partition_broadcast(bias_bc[:], bias_row[0:1, :])

        xT = big.tile([P, CT, NTOK], BF16)     # x.T  (feature-major x)
        wT = big.tile([P, CT, 3 * C], BF16)    # qkv_w.T
        pwT = big.tile([P, CT, C], BF16)       # proj_w.T
        pg = big.tile([P, CT, C], F32)         # proj_w fp32 (transposed pair 4)

        # ---- startup: load + cast + transpose x and qkv_w (bf16 path) ----
        with tc.tile_pool(name="staging", bufs=2) as staging, \
                tc.tile_pool(name="ps_tp", bufs=2, space="PSUM") as ps_tp:

            # HAM warmup: keep the PE busy during the initial input-DMA wait
            # so its clock gate opens (1.2 -> 2.4 GHz) before the real
            # transpose and qkv stream begins.  identb.T == identb, and
            # writing it back makes the chain live (not DCE-able) and orders
            # warmup before first real use.
            warm_ps = ps_tp.tile([P, 1024], BF16, tag="tp")
            for i in range(32):
                nc.tensor.transpose(warm_ps[:, :P], identb[:], identb[:])
            nc.vector.tensor_copy(identb[:], warm_ps[:, :P])

            def transpose_blocks_bf16(srcs, dst_full):
                """srcs: list of [128,128] bf16 SBUF APs; dst_full:
                [128, len*128] bf16 AP, contiguous. PE-transpose each block
                (bf16: 1 cyc/col), copy out in groups of up to 8 (one PSUM
                bank; 16-bit packed DVE copy amortizes)."""
                i = 0
                while i < len(srcs):
                    n = min(8, len(srcs) - i)
                    pt = ps_tp.tile([P, 1024], BF16, tag="tp")
                    for j in range(n):
                        nc.tensor.transpose(
                            pt[:, j * P:(j + 1) * P], srcs[i + j], identb[:]
                        )
                    nc.vector.tensor_copy(
                        dst_full[:, i * P:(i + n) * P], pt[:, : n * P]
                    )
                    i += n

            # x group 0, then the two w groups holding q/k weights, then the
            # second x group, then v weights: gets pair-0 q/k built earliest
            def emit_xg(g):
                xg = staging.tile([P, CT, C], F32, tag="g", name=f"xg{g}")
                xgb = staging.tile([P, CT, C], BF16, tag="gb", name=f"xgb{g}")
                for j in range(5):
                    t0 = (g * 5 + j) * P
                    nc.sync.dma_start(xg[:, j, :], x_ext.ap()[t0:t0 + P, :])
                    nc.vector.tensor_copy(xgb[:, j, :], xg[:, j, :])
                for ct in range(CT):
                    transpose_blocks_bf16(
                        [xgb[:, j, ct * P:(ct + 1) * P] for j in range(5)],
                        xT[:, ct, g * 5 * P:(g * 5 + 5) * P],
                    )

            def emit_wg(g):
                wg = staging.tile([P, CT, C], F32, tag="g", name=f"wg{g}")
                wgb = staging.tile([P, CT, C], BF16, tag="gb", name=f"wgb{g}")
                for j in range(6):
                    f0 = (g * 6 + j) * P
                    nc.sync.dma_start(wg[:, j, :], w_ext.ap()[f0:f0 + P, :])
                    nc.vector.tensor_copy(wgb[:, j, :], wg[:, j, :])
                for ct in range(CT):
                    transpose_blocks_bf16(
                        [wgb[:, j, ct * P:(ct + 1) * P] for j in range(6)],
                        wT[:, ct, g * 6 * P:(g * 6 + 6) * P],
                    )

            emit_xg(0)
            emit_wg(0)
            emit_wg(1)
            emit_xg(1)
            emit_wg(2)
            # ---- proj_w: DMA now, transpose later (filler work in pair 4,
            # through the fp32 filler ring) ----
            for j in range(CT):
                nc.sync.dma_start(pg[:, j, :], pw_ext.ap()[j * P:(j + 1) * P, :])

        big2 = ctx.enter_context(tc.tile_pool(name="big2", bufs=1))
        # q (scaled) and k, feature-major, 2-slot rotation keyed by pair%2
        qk = big2.tile([P, 2, 2, NTOK], BF16)
        v_sb = big2.tile([P, TT, H, P], BF16)  # [1 | 63 zeros | v] per head
        ot_all = big2.tile([P, CT, NTOK], BF16)     # attention out, feature-major
        out_sb = big2.tile([P, TT, C], F32)

        # v_aug layout per head: col 0 = ones (softmax denominator row),
        # cols 1:64 = zeros (padding so O lands at partitions 64:128)
        nc.gpsimd.memset(v_sb[:, :, :, 0:64], 0.0)
        nc.gpsimd.memset(v_sb[:, :, :, 0:1], 1.0)

        ps_fill = ctx.enter_context(tc.tile_pool(name="ps_fill", bufs=2, space="PSUM"))

        def transpose_blocks_f32(srcs, dst_full):
            """fp32 transpose path via the filler ring (used only for proj_w,
            36 blocks: not worth a dedicated bf16 staging)."""
            i = 0
            while i < len(srcs):
                n = min(4, len(srcs) - i)
                pt = ps_fill.tile([P, 512], F32, tag="fill")
                for j in range(n):
                    nc.tensor.transpose(
                        pt[:, j * P:(j + 1) * P], srcs[i + j], ident_f()
                    )
                nc.vector.tensor_copy(
                    dst_full[:, i * P:(i + n) * P], pt[:, : n * P]
                )
                i += n

        # fp32 identity for the proj_w transposes (made lazily, as filler)
        _identf = [None]

        def ident_f():
            if _identf[0] is None:
                _identf[0] = const.tile([P, P], F32, name="identf")
                make_identity(nc, _identf[0])
            return _identf[0]

        # ---- qkv projection (emitted interleaved with attention below) ----
        def emit_qk_chunk(hp, which, c0, cw):
            """qk[slot, which] = (q|k) row block of head pair hp,
            feature-major, for token chunk [c0, c0+cw)."""
            ft = hp + 6 * which
            ps = ps_fill.tile([P, 512], F32, tag="fill", name=f"qkp{ft}_{c0}")
            for ct in range(CT):
                nc.tensor.matmul(
                    ps[:, :cw],
                    wT[:, ct, ft * P:(ft + 1) * P],
                    xT[:, ct, c0:c0 + cw],
                    start=(ct == 0), stop=(ct == CT - 1),
                )
            if which == 0:  # q: fold in softmax scale
                nc.vector.tensor_scalar_mul(
                    qk[:, hp % 2, 0, c0:c0 + cw], ps[:, :cw], SCALE
                )
            else:
                nc.vector.tensor_copy(qk[:, hp % 2, 1, c0:c0 + cw], ps[:, :cw])

        def qk_pair_chunks(p):
            # q/k interleaved so the chunks a consumer needs first come out
            # adjacent; template needs both c0 chunks only
            return [(p, w, c0, cw)
                    for c0, cw in ((0, 512), (512, 512), (1024, 256))
                    for w in (0, 1)]

        # v token-major: v[tok, f] = x @ qkv_w.T cols [1536, 2304)
        def emit_v_chunk(tt, half):
            c0, cw, h0, nh = ((0, 512, 0, 8), (512, 256, 8, 4))[half]
            ps = ps_fill.tile([P, 512], F32, tag="fill", name=f"vp{tt}_{half}")
            for ct in range(CT):
                nc.tensor.matmul(
                    ps[:, :cw],
                    xT[:, ct, tt * P:(tt + 1) * P],
                    wT[:, ct, 2 * C + c0:2 * C + c0 + cw],
                    start=(ct == 0), stop=(ct == CT - 1),
                )
            nc.vector.tensor_copy(
                v_sb[:, tt, h0:h0 + nh, 64:128],
                ps[:, :cw].rearrange("p (h e) -> p h e", e=HD),
            )

        # ---- output projection ----
        def emit_proj_chunk(tt, half):
            c0, cw = ((0, 512), (512, 256))[half]
            ps = ps_fill.tile([P, 512], F32, tag="fill", name=f"prj{tt}_{c0}")
            for ct in range(CT):
                nc.tensor.matmul(
                    ps[:, :cw],
                    ot_all[:, ct, tt * P:(tt + 1) * P],
                    pwT[:, ct, c0:c0 + cw],
                    start=(ct == 0), stop=(ct == CT - 1),
                )
            nc.vector.tensor_tensor(
                out_sb[:, tt, c0:c0 + cw], ps[:, :cw],
                bias_bc[:, c0:c0 + cw], ADD,
            )
            if half == 1:
                nc.sync.dma_start(out_ext.ap()[tt * P:(tt + 1) * P, :],
                                  out_sb[:, tt, :])

        def emit_filler(kind, arg):
            if kind == "qk":
                emit_qk_chunk(*arg)
            elif kind == "v":
                emit_v_chunk(*arg)
            elif kind == "proj":
                emit_proj_chunk(*arg)
            else:  # "pw": deferred proj_w transpose for channel tile arg
                transpose_blocks_f32(
                    [pg[:, j, arg * P:(arg + 1) * P] for j in range(CT)],
                    pwT[:, arg, :],
                )

        # q/k for head pair 0 up front
        for a in qk_pair_chunks(0):
            emit_qk_chunk(*a)

        # only the first two token tiles of v are needed before pair 0 starts
        # (template + first search units); the rest stream as pair-0 filler
        for tt in (0, 1):
            emit_v_chunk(tt, 0)
            emit_v_chunk(tt, 1)

        # ---- attention ----
        ps_sc = ctx.enter_context(tc.tile_pool(name="ps_sc", bufs=2, space="PSUM"))
        ps_ot = ctx.enter_context(tc.tile_pool(name="ps_ot", bufs=2, space="PSUM"))
        pts = ctx.enter_context(tc.tile_pool(name="pts", bufs=4))
        dn = ctx.enter_context(tc.tile_pool(name="dn", bufs=2))
        rbp = ctx.enter_context(tc.tile_pool(name="rbp", bufs=2))

        def qh(h, c0, cw):
            b = (h % 2) * 64
            return qk[b:b + 64, (h // 2) % 2, 0, c0:c0 + cw]

        def kh(h, tk):
            b = (h % 2) * 64
            return qk[b:b + 64, (h // 2) % 2, 1, tk * P:(tk + 1) * P]

        def normalize(h, ot_ps, c0, cw):
            """ot_ps: [128, cw] psum (row 0 = denominators, rows 64:128 = O.T
            for tq cols [c0, c0+cw)). Normalize and write to ot_all, fully off
            the ACT queue (one wide DVE copy lifts PSUM->SBUF so the PSUM
            slot frees early)."""
            b = (h % 2) * 64
            den = dn.tile([P, 512], F32, tag="dn")
            nc.vector.tensor_copy(den[:, :cw], ot_ps[:, :cw])
            rb = rbp.tile([P, 512], F32, tag="rb")
            nc.gpsimd.partition_broadcast(rb[:, :cw], den[0:1, :cw])
            # approx reciprocal (~18 bits, plenty for bf16 outputs)
            nc.vector.reciprocal_approx_fast(rb[:, :cw], rb[:, :cw])
            nc.vector.tensor_tensor(
                ot_all[b:b + 64, h // 2, c0:c0 + cw],
                den[64:128, :cw], rb[64:128, :cw], MULT,
            )

        for hp in range(6):
            h0, h1 = 2 * hp, 2 * hp + 1
            # filler work fed into PE idle slots while the ACT-bound
            # attention runs, split across the two cj chunks.
            # pair 0 carries the v token tiles 2..9 (JIT ahead of their AV
            # use in cj 0); pair 4 carries the deferred proj_w transposes;
            # pair 5 carries the early proj tiles (set after cj-0 normalize).
            if hp == 0:
                pend = [[("v", (tt, half)) for tt in range(2, TT)
                         for half in (0, 1)],
                        [("qk", a) for a in qk_pair_chunks(1)]]
            elif hp < 4:
                nxt = [("qk", a) for a in qk_pair_chunks(hp + 1)]
                pend = [nxt[:3], nxt[3:]]
            elif hp == 4:
                pend = [[("qk", a) for a in qk_pair_chunks(5)],
                        [("pw", ct) for ct in range(CT)]]
            else:
                pend = [[], []]  # cj1 list filled after cj0 normalize

            # template block, both heads fused: queries [0,256) x keys [0,256)
            st_t = ps_sc.tile([P, 1024], F32, tag="sc", name=f"tst{hp}")
            for tj in range(2):
                for hi, h in enumerate((h0, h1)):
                    nc.tensor.matmul(
                        st_t[:, hi * 512 + tj * NT: hi * 512 + (tj + 1) * NT],
                        kh(h, tj), qh(h, 0, NT), start=True, stop=True,
                    )
            pt_t = pts.tile([P, 1024], BF16, tag="pt", name=f"tpt{hp}")
            nc.scalar.activation(pt_t[:], st_t[:], EXP)
            for hi, h in enumerate((h0, h1)):
                to = ps_fill.tile([P, 512], F32, tag="fill", name=f"to{h}")
                for tj in range(2):
                    nc.tensor.matmul(
                        to[:, :NT], v_sb[:, tj, h, :],
                        pt_t[:, hi * 512 + tj * NT: hi * 512 + (tj + 1) * NT],
                        start=(tj == 0), stop=(tj == 1),
                    )
                normalize(h, to, 0, NT)

            # search: queries [256, 1280) attend all keys, cj-outer
            for cj in range(2):
                c0 = NT + cj * 512
                pending = pend[cj]
                ots = {h: ps_ot.tile([P, 512], F32, tag="ot",
                                     name=f"ot{h}_{cj}")
                       for h in (h0, h1)}
                for tk in range(TT):
                    st = ps_sc.tile([P, 1024], F32, tag="sc",
                                    name=f"st{hp}_{cj}_{tk}")
                    # the two heads run CONCURRENTLY on PE row groups
                    # 0-63 / 64-127, filling the two halves of one tile
                    for hi, h in enumerate((h0, h1)):
                        nc.tensor.matmul(
                            st[:, hi * 512:(hi + 1) * 512],
                            kh(h, tk), qh(h, c0, 512), start=True, stop=True,
                        )
                    pt = pts.tile([P, 1024], BF16, tag="pt",
                                  name=f"pt{hp}_{cj}_{tk}")
                    nc.scalar.activation(pt[:], st[:], EXP)
                    for hi, h in enumerate((h0, h1)):
                        nc.tensor.matmul(
                            ots[h][:, :], v_sb[:, tk, h, :],
                            pt[:, hi * 512:(hi + 1) * 512],
                            start=(tk == 0), stop=(tk == TT - 1),
                        )
                    # feed filler into the PE stream (pair 0 cj 0 carries the
                    # v tail and needs a higher drain rate to stay JIT-ahead
                    # of its AV consumers)
                    for _ in range(2 if (hp == 0 and cj == 0) else 1):
                        if pending:
                            emit_filler(*pending.pop(0))
                for h in (h0, h1):
                    normalize(h, ots[h], c0, 512)
                while pending:
                    emit_filler(*pending.pop(0))
                if hp == 5 and cj == 0:
                    # queries 256-767 (token tiles 2-5) are final once every
                    # pair's cj-0 normalize is done -> their proj overlaps
                    # the cj-1 attention as filler
                    pend[1] = [("proj", (tt, half)) for tt in (2, 3, 4, 5)
                               for half in (0, 1)]

        # remaining output projection (template tiles + cj-1 tiles)
        for tt in (0, 1, 6, 7, 8, 9):
            for half in (0, 1):
                emit_proj_chunk(tt, half)

    nc.compile()
    return nc


_NC = None


def _get_nc():
    global _NC
    if _NC is None:
        _NC = build_nc()
    return _NC


def kernel(x, qkv_w, proj_w, proj_b, **_ignored):
    from concourse.bass_utils import run_bass_kernel_spmd

    x = np.ascontiguousarray(np.asarray(x), dtype=np.float32)
    qkv_w = np.ascontiguousarray(np.asarray(qkv_w), dtype=np.float32)
    proj_w = np.ascontiguousarray(np.asarray(proj_w), dtype=np.float32)
    proj_b = np.ascontiguousarray(np.asarray(proj_b), dtype=np.float32).reshape(1, C)

    nc = _get_nc()
    in_maps = [
        {"x": x[i], "qkv_w": qkv_w, "proj_w": proj_w, "proj_b": proj_b}
        for i in range(8)
    ]
    res = run_bass_kernel_spmd(nc, in_maps, list(range(8)))
    return np.stack([res.results[i]["out"] for i in range(8)])


if __name__ == "__main__":
    rng = np.random.default_rng(0)
    ins = {
        "x": rng.standard_normal((8, NTOK, C), dtype=np.float32),
        "qkv_w": rng.standard_normal((3 * C, C), dtype=np.float32) * 0.02,
        "proj_w": rng.standard_normal((C, C), dtype=np.float32) * 0.02,
        "proj_b": np.zeros(C, dtype=np.float32),
    }
    out = kernel(**ins)
    print("out", out.shape, out.dtype)
